# revision 23
# baseline (speedup 1.0000x reference)
# BatchGAT Trainium2 Bass kernel — bucketed threshold-sum formulation.
#
# Reference computation (per batch b, head hd):
#   hp = h[b] @ w[hd]; t = tanh(hp)
#   s = t @ a_src[hd]; d = t @ a_dst[hd]
#   attn[i,j] = softmax_j(leaky_relu(s[i] + d[j], 0.2))
#   out = attn @ hp + bias_p
#
# Softmax_j is invariant to a per-i scale; multiplying by exp(-0.2 s_i)
# gives numerator terms max(e^{0.8 s_i} e^{d_j}, e^{0.2 d_j}) whose branch
# choice depends only on the ORDER of d_j vs -s_i. Quantizing d onto 127
# monotone buckets turns the n^2 attention sum into small bucket tables:
#   T1[k] = sum_{q(d_j)=k} e^{d_j} hp_ext[j],  T2[k] = sum e^{0.2 d_j} hp_ext[j]
#   num[i] = e^{0.8 s_i} * sum_{k>=t_i} T1[k] + Tot2 - sum_{k>=t_i} T2[k]
#   out[i] = num[i][:64] / num[i][64]        (hp_ext = [hp | 1], t_i = q(-s_i))
# T1/T2 ride one [128,130] table whose row 127 holds -Tot2 so a single
# step-mask matmul per i-tile yields [G1 | G2-Tot2]. All masks and the
# combine are batched n-wide DVE ops (stride-0 broadcast APs); instruction
# count per (b,h) pair is ~60 vs ~350 for the direct n^2 kernel.
#
# Sharding: head-parallel, one head per NeuronCore; each core does all 4
# batches of its head. h ships pre-transposed bf16 [b, 64, n]; bias_p is
# added on the host (out = attn@hp + b exactly).

import numpy as np
import ml_dtypes
from contextlib import ExitStack

import concourse.bass as bass
import concourse.tile as tile
import concourse.mybir as mybir
from concourse import bacc
from concourse.bass_utils import run_bass_kernel_spmd

F32 = mybir.dt.float32
BF16 = mybir.dt.bfloat16
I32 = mybir.dt.int32
AF = mybir.ActivationFunctionType
ALU = mybir.AluOpType

NB = 4      # batches
NF = 64     # f_in == f_out
NH = 8      # heads == cores
NBUCK = 128          # mask/table width; buckets 0..126, row 127 = -Tot2
KMAX = float(NBUCK - 2)
DLO, DHI = -5.0, 5.0
DELTA = (DHI - DLO) / NBUCK
RND = 8388608.0      # 2^23: x+RND-RND rounds f32 to nearest int
NW = 130             # combined table width: [T1(65) | T2(65)]


def _chunks(total, size):
    out = []
    c0 = 0
    while c0 < total:
        cs = min(size, total - c0)
        out.append((c0, cs))
        c0 += cs
    return out


def _rep0(ap_src, inner):
    # stride-0 inner broadcast: [128, m] -> [128, m, inner]
    return bass.AP(tensor=ap_src.tensor, offset=ap_src.offset,
                   ap=[list(ap_src.ap[0])] + [list(p) for p in ap_src.ap[1:]]
                   + [[0, inner]])


def build_gat_module(n=2048, nb=NB):
    nc = bacc.Bacc("TRN2", target_bir_lowering=False)

    ht_t = nc.dram_tensor("ht", [nb, NF, n], BF16, kind="ExternalInput")
    w_t = nc.dram_tensor("w1", [NF, NF], F32, kind="ExternalInput")
    asd_t = nc.dram_tensor("asd", [NF, 2], F32, kind="ExternalInput")
    o_t = nc.dram_tensor("out", [nb, n, NF], F32, kind="ExternalOutput")

    NT = n // 128
    C512 = _chunks(n, 512)
    nw = len(C512)
    WAVE = 6                     # gather wave size (psum banks: 6*256*4B = 3)

    with tile.TileContext(nc) as tc:
        with ExitStack() as ctx:
            consts = ctx.enter_context(tc.tile_pool(name="consts", bufs=1))
            hpool = ctx.enter_context(tc.tile_pool(name="hpool", bufs=1))
            work = ctx.enter_context(tc.tile_pool(name="work", bufs=6))
            pairbuf = ctx.enter_context(tc.tile_pool(name="pairbuf", bufs=2))
            outp = ctx.enter_context(tc.tile_pool(name="outp", bufs=2))
            pst = ctx.enter_context(tc.tile_pool(name="pst", bufs=1, space="PSUM"))
            pacc = ctx.enter_context(tc.tile_pool(name="pacc", bufs=1, space="PSUM"))
            drampool = ctx.enter_context(
                tc.tile_pool(name="drampool", bufs=2, space="DRAM"))

            # ---- constants ----
            from concourse.masks import make_identity
            ident_bf = consts.tile([128, 128], BF16)
            make_identity(nc, ident_bf)
            w_f32 = consts.tile([128, NF], F32)
            nc.sync.dma_start(out=w_f32[0:NF, :], in_=w_t[:, :])
            nc.sync.dma_start(out=w_f32[NF:128, :], in_=w_t[:, :])
            w_sb = consts.tile([128, NF], BF16)
            nc.vector.tensor_copy(w_sb, w_f32)
            asd_f32 = consts.tile([NF, 2], F32)
            nc.sync.dma_start(out=asd_f32, in_=asd_t[:, :])
            asd_sb = consts.tile([NF, 2], BF16)
            nc.vector.tensor_copy(asd_sb, asd_f32)
            iota_i32 = consts.tile([128, NBUCK], I32)
            nc.gpsimd.iota(iota_i32, pattern=[[1, NBUCK]], base=0,
                           channel_multiplier=0)
            iota_row = consts.tile([128, NBUCK], BF16)
            nc.vector.tensor_copy(iota_row, iota_i32)
            iotac_i32 = consts.tile([128, 1], I32)
            nc.gpsimd.iota(iotac_i32, pattern=[[0, 1]], base=0,
                           channel_multiplier=1)
            iota_colf = consts.tile([128, 1], F32)
            nc.vector.tensor_copy(iota_colf, iotac_i32)
            negones = consts.tile([128, 1], BF16)
            nc.vector.memset(negones, -1.0)

            # ---- load pre-transposed h ----
            nhalf = nb // 2
            hTT = []
            for half in range(nhalf):
                row = []
                for q, (c0, cs) in enumerate(C512):
                    t_q = hpool.tile([128, 512], BF16, name=f"hTT{half}_{q}")
                    nc.sync.dma_start(
                        out=t_q[0:NF, 0:cs], in_=ht_t[2 * half, :, c0:c0 + cs])
                    nc.sync.dma_start(
                        out=t_q[NF:128, 0:cs],
                        in_=ht_t[2 * half + 1, :, c0:c0 + cs])
                    row.append(t_q)
                hTT.append(row)

            def stage1(b):
                half, bp = b // 2, NF * (b % 2)
                hTq = [hTT[half][q][bp:bp + NF, :] for q in range(nw)]
                w_b = w_sb[bp:bp + NF, :]
                st = {}

                # B: T = tanh(w.T @ hT) row layout [64, n]
                T_sb = pairbuf.tile([NF, n], BF16, name="T_sb")
                for icx, (c0, cs) in enumerate(C512):
                    psB = pst.tile([NF, 512], F32, name="psB")
                    mi = nc.tensor.matmul(
                        psB[:, 0:cs], lhsT=w_b, rhs=hTq[icx][:, 0:cs],
                        start=True, stop=True)
                    if icx > 0:
                        mi.ins.ldweights = False
                    nc.scalar.activation(
                        T_sb[:, c0:c0 + cs], psB[:, 0:cs], AF.Tanh)

                # D: s,d columns via psD[:, jb, 0|1]
                psD = pacc.tile([128, NT, 2], F32, name="psD")
                for jb in range(NT):
                    nc.tensor.matmul(
                        psD[:, jb, :],
                        lhsT=T_sb[:, jb * 128:(jb + 1) * 128],
                        rhs=asd_sb, start=True, stop=True)
                e8s_col = pairbuf.tile([128, NT], F32, name="e8s_col")
                nc.scalar.activation(e8s_col, psD[:, :, 0], AF.Exp, scale=0.8)
                ed_col = pairbuf.tile([128, NT], F32, name="ed_col")
                nc.scalar.activation(ed_col, psD[:, :, 1], AF.Exp)
                ed2_col = pairbuf.tile([128, NT], F32, name="ed2_col")
                nc.scalar.activation(ed2_col, psD[:, :, 1], AF.Exp, scale=0.2)
                st["e8s_col"] = e8s_col

                # bucket(d_j) column: round+clip((d - DLO)/DELTA) to [0,126]
                qd = work.tile([128, NT], F32, name="qd")
                nc.scalar.mul(qd, psD[:, :, 1], 1.0 / DELTA)
                rd = work.tile([128, NT], F32, name="rd")
                nc.vector.tensor_scalar(
                    out=rd, in0=qd, scalar1=RND - DLO / DELTA, scalar2=RND,
                    op0=ALU.add, op1=ALU.subtract)
                kd_col = pairbuf.tile([128, NT], F32, name="kd_col")
                nc.vector.tensor_scalar(
                    out=kd_col, in0=rd, scalar1=0.0, scalar2=KMAX,
                    op0=ALU.max, op1=ALU.min)

                # threshold bucket t_i = q(-s_i): col -> row via DVE
                # transpose -> DRAM roundtrip broadcast
                qs = work.tile([128, NT], F32, name="qs")
                nc.scalar.mul(qs, psD[:, :, 0], -1.0 / DELTA)
                rs = work.tile([128, NT], F32, name="rs")
                nc.vector.tensor_scalar(
                    out=rs, in0=qs, scalar1=RND - DLO / DELTA, scalar2=RND,
                    op0=ALU.add, op1=ALU.subtract)
                bn_col = work.tile([128, NT], BF16, name="bn_col")
                nc.vector.tensor_scalar(
                    out=bn_col, in0=rs, scalar1=0.0, scalar2=KMAX,
                    op0=ALU.max, op1=ALU.min)
                psTr = pacc.tile([NT, 128], BF16, name="psTr")
                nc.tensor.transpose(psTr, bn_col, ident_bf)
                bn_row = work.tile([NT, 128], BF16, name="bn_row")
                nc.scalar.copy(bn_row, psTr)
                bn_dram = drampool.tile([NT, 128], BF16, name="bn_dram")
                nc.sync.dma_start(out=bn_dram, in_=bn_row)
                bn_bc = pairbuf.tile([128, n], BF16, name="bn_bc")
                bdap = bn_dram[0, 0:128]
                for (c0, cs) in C512:
                    nc.sync.dma_start(out=bn_bc[:, c0:c0 + cs], in_=bass.AP(
                        tensor=bdap.tensor, offset=bdap.offset + c0,
                        ap=[[0, 128], [1, cs]]))

                # A: hp_ext[:, jb, 0:64] = hp rows, col 64 = 1.0
                hp_ext = pairbuf.tile([128, NT, 66], BF16, name="hp_ext")
                nc.vector.memset(hp_ext[:, :, 64:65], 1.0)
                for (j0, js) in _chunks(NT, 8):
                    psA = pst.tile([128, min(8, NT), NF], F32, name="psA")
                    for k in range(js):
                        jb = j0 + k
                        nc.tensor.matmul(
                            psA[:, k, :],
                            lhsT=hTq[jb // 4][:, (jb % 4) * 128:
                                              (jb % 4 + 1) * 128],
                            rhs=w_b, start=True, stop=True)
                    nc.scalar.copy(hp_ext[:, j0:j0 + js, 0:NF], psA[:, 0:js, :])

                # values: edhp_all = [ed*hp_ext | ed2*hp_ext]  (one tile)
                edhp_all = pairbuf.tile([128, NT, NW], BF16, name="edhp_all")
                nc.vector.tensor_tensor(
                    out=edhp_all[:, :, 0:65], in0=hp_ext[:, :, 0:65],
                    in1=_rep0(ed_col[:, :], 65), op=ALU.mult)
                nc.vector.tensor_tensor(
                    out=edhp_all[:, :, 65:130], in0=hp_ext[:, :, 0:65],
                    in1=_rep0(ed2_col[:, :], 65), op=ALU.mult)
                st["edhp_all"] = edhp_all

                # masks: onehot_all[j, jb, k] = (kd[j,jb] == k)
                onehot_all = pairbuf.tile([128, NT, NBUCK], BF16,
                                          name="onehot_all")
                iap = iota_row[:, :]
                nc.vector.tensor_tensor(
                    out=onehot_all, in0=_rep0(kd_col[:, :], NBUCK),
                    in1=bass.AP(tensor=iap.tensor, offset=iap.offset,
                                ap=[list(iap.ap[0]), [0, NT], [1, NBUCK]]),
                    op=ALU.is_equal)
                st["onehot_all"] = onehot_all

                # step mask: hge_all[k, i] = (t_i <= k)
                hge_all = pairbuf.tile([128, n], BF16, name="hge_all")
                nc.vector.tensor_scalar(
                    out=hge_all, in0=bn_bc, scalar1=iota_colf, scalar2=None,
                    op0=ALU.is_le)
                st["hge_all"] = hge_all
                return st

            def stageF(st):
                # scatter into combined table, then -Tot2 into row 127.
                # PE psum writes must start at partition 0/32/64, so -Tot2
                # lands in spare cols at partition 0 and a tiny SBUF->SBUF
                # DMA hops it across partitions into row 127.
                psT12 = pacc.tile([128, 196], F32, name="psT12")
                for jb in range(NT):
                    nc.tensor.matmul(
                        psT12[:, 0:NW], lhsT=st["onehot_all"][:, jb, :],
                        rhs=st["edhp_all"][:, jb, :],
                        start=(jb == 0), stop=(jb == NT - 1))
                T12_sb = pairbuf.tile([128, NW], BF16, name="T12_sb")
                nc.scalar.copy(T12_sb, psT12[:, 0:NW])
                nc.tensor.matmul(
                    psT12[0:1, 130:195], lhsT=negones[0:127, 0:1],
                    rhs=T12_sb[0:127, 65:130], start=True, stop=True,
                    skip_group_check=True)
                totrow = work.tile([1, 65], BF16, name="totrow")
                nc.scalar.copy(totrow, psT12[0:1, 130:195])
                nc.sync.dma_start(out=T12_sb[127:128, 65:130], in_=totrow)
                st["T12_sb"] = T12_sb
                return st

            def stageG(st, b):
                o_full = outp.tile([128, NT, NF], F32, name="o_full")
                for w0 in range(0, NT, WAVE):
                    ws = min(WAVE, NT - w0)
                    psG = pacc.tile([128, WAVE, 256], F32, name="psG")
                    for k in range(ws):
                        it = w0 + k
                        nc.tensor.matmul(
                            psG[:, k, 0:NW],
                            lhsT=st["hge_all"][:, it * 128:(it + 1) * 128],
                            rhs=st["T12_sb"], start=True, stop=True)
                    # tmp = e8s*G1 ; numn = (G2-Tot2) - tmp = -num
                    tmp = work.tile([128, WAVE, 66], F32, name="tmp")
                    e8ap = st["e8s_col"][:, w0:w0 + ws]
                    nc.vector.tensor_tensor(
                        out=tmp[:, 0:ws, 0:65], in0=psG[:, 0:ws, 0:65],
                        in1=_rep0(e8ap, 65), op=ALU.mult)
                    numn = work.tile([128, WAVE, 66], F32, name="numn")
                    nc.vector.tensor_tensor(
                        out=numn[:, 0:ws, 0:65], in0=psG[:, 0:ws, 65:130],
                        in1=tmp[:, 0:ws, 0:65], op=ALU.subtract)
                    r = work.tile([128, WAVE], F32, name="r")
                    nc.vector.reciprocal(r[:, 0:ws], numn[:, 0:ws, 64:65])
                    # out = (-num)*(-1/den) on gpsimd (idle engine)
                    nc.gpsimd.tensor_tensor(
                        out=o_full[:, w0:w0 + ws, :], in0=numn[:, 0:ws, 0:64],
                        in1=_rep0(r[:, 0:ws], NF), op=ALU.mult)
                oap = o_t[b, :, :]
                nc.sync.dma_start(
                    out=bass.AP(tensor=oap.tensor, offset=oap.offset,
                                ap=[[NF, 128], [128 * NF, NT], [1, NF]]),
                    in_=o_full)

            prev = None
            for b in range(nb):
                st = stage1(b)
                if prev is not None:
                    stageG(prev[0], prev[1])
                stageF(st)
                prev = (st, b)
            stageG(prev[0], prev[1])

    nc.compile()
    return nc


_CACHE = {}
_last_results = None


def _get_nc(n=2048, nb=NB):
    key = (n, nb)
    if key not in _CACHE:
        _CACHE[key] = build_gat_module(n, nb)
    return _CACHE[key]


def kernel(h, adj, w, a_src, a_dst, bias_p):
    global _last_results
    h = np.asarray(h, dtype=np.float32)
    w = np.asarray(w, dtype=np.float32)
    a_src = np.asarray(a_src, dtype=np.float32)
    a_dst = np.asarray(a_dst, dtype=np.float32)
    bias_p = np.asarray(bias_p, dtype=np.float32)
    nb, n, _ = h.shape

    ht = np.ascontiguousarray(
        np.transpose(h, (0, 2, 1))).astype(ml_dtypes.bfloat16)

    nc = _get_nc(n, nb)
    in_maps = []
    for c in range(NH):
        asd = np.ascontiguousarray(
            np.concatenate([a_src[c], a_dst[c]], axis=1).astype(np.float32))
        in_maps.append({
            "ht": ht,
            "w1": np.ascontiguousarray(w[c]),
            "asd": asd,
        })
    res = run_bass_kernel_spmd(nc, in_maps, core_ids=list(range(NH)))
    _last_results = res
    out = np.empty((nb, NH, n, NF), np.float32)
    for c in range(NH):
        out[:, c] = res.results[c]["out"]
    # bias applied on host: out = attn@hp + bias (exact)
    out += bias_p[None, None, None, :]
    return out
